# revision 10
# baseline (speedup 1.0000x reference)
"""Multi-head attention Trainium2 kernel (8-core SPMD, head-parallel).

Problem: nn_MultiHeadAttention (B=2, S=2048, d_model=1024, H=16, d_k=64).

Sharding: heads are split across the 8 cores (2 heads x 2 batches per core).
Each core holds the column block of W_q/W_k/W_v for its 2 heads and the
matching row block of W_o.T, computes a full [4096, 1024] partial of the
output projection, and the host sums the 8 partials (the "all-reduce").

Per-core layout strategy (everything keyed to keep the contraction dim on
SBUF partitions with contiguous DMA):
  - Host ships X.T = [1024, 4096] bf16 per input, so projection matmuls read
    both operands naturally.  Q/K are produced transposed (Q.T = [128, 4096]),
    which is exactly the layout the scores matmul wants.
  - Scores are computed transposed: S.T[k, q] = K_h Q_h.T, so the attention
    weight matrix lands with k on partitions, exp applied on eviction
    (ScalarE), and the AV matmul consumes it directly as the moving operand
    with V in natural [token, d] layout as the stationary operand.
  - Softmax denominators come for free by appending a ones column to V
    (M=65): row 64 (head A) / row 0 (head B) of the AV accumulator is
    sum_k exp(s).  Softmax max-subtraction is skipped: scores are ~N(0,1) so
    exp never overflows, and softmax is shift-invariant.
  - Normalization is folded in after the AV stage: reciprocal of the
    denominators, GPSIMD partition-broadcast, one elementwise multiply.
"""

import os
import sys

sys.path.insert(0, "/opt/trn_rl_repo")

import numpy as np
import ml_dtypes

import concourse.bass as bass
import concourse.mybir as mybir
import concourse.tile as tile
import concourse.bacc as bacc
from concourse.bass_utils import run_bass_kernel_spmd
from concourse.masks import make_identity

BF16 = mybir.dt.bfloat16
F32 = mybir.dt.float32
NP_BF16 = ml_dtypes.bfloat16

B, S, D = 2, 2048, 1024
H, DK = 16, 64
T = B * S            # 4096 tokens
N_CORES = 8
HPC = H // N_CORES   # heads per core = 2
HD = HPC * DK        # 128 per-core head dims
KT = D // 128        # 8 contraction tiles for projections
TCH = 512            # token chunk for projections
QC = 512             # q chunk for attention
NKT = S // 128       # 16 k tiles per batch
VEXT_W = 130         # per-k-tile width in v_ext: [V_A(64)|1][1|V_B(64)]
SCALE = 1.0 / np.sqrt(DK)


def build_nc():
    nc = bacc.Bacc("TRN2", target_bir_lowering=False, debug=False,
                   num_devices=N_CORES)

    xq_t = nc.dram_tensor("xq_t", [D, T], BF16, kind="ExternalInput").ap()
    xk_t = nc.dram_tensor("xk_t", [D, T], BF16, kind="ExternalInput").ap()
    xv_t = nc.dram_tensor("xv_t", [D, T], BF16, kind="ExternalInput").ap()
    wq_t = nc.dram_tensor("wq_t", [D, HD], BF16, kind="ExternalInput").ap()
    wk_t = nc.dram_tensor("wk_t", [D, HD], BF16, kind="ExternalInput").ap()
    wv_t = nc.dram_tensor("wv_t", [D, HD], BF16, kind="ExternalInput").ap()
    wo = nc.dram_tensor("wo", [HD, D], BF16, kind="ExternalInput").ap()
    out_p = nc.dram_tensor("out_p", [T, D], F32, kind="ExternalOutput").ap()

    with tile.TileContext(nc) as tc, \
         tc.tile_pool(name="persist", bufs=1) as persist:
        # ---- persistent SBUF tensors (one slot per tag) ------------------
        def ptile(shape, dtype, name):
            return persist.tile(shape, dtype, name=name, tag=name)

        wq_sb = ptile([128, KT * HD], BF16, "wq_sb")
        wk_sb = ptile([128, KT * HD], BF16, "wk_sb")
        wv_sb = ptile([128, KT * HD], BF16, "wv_sb")
        wo_sb = ptile([128, D], BF16, "wo_sb")
        qt_sb = ptile([128, T], BF16, "qt_sb")
        kt_sb = ptile([128, T], BF16, "kt_sb")
        vt_sb = ptile([128, T], BF16, "vt_sb")
        vext_sb = ptile([128, (T // 128) * VEXT_W], BF16, "vext_sb")
        ot_sb = ptile([128, T], BF16, "ot_sb")
        ident_sb = ptile([128, 128], BF16, "ident_sb")

        # ---- weight loads + constants ------------------------------------
        for w_sb, w_dram in ((wq_sb, wq_t), (wk_sb, wk_t), (wv_sb, wv_t)):
            for j in range(KT):
                nc.sync.dma_start(w_sb[:, j * HD:(j + 1) * HD],
                                  w_dram[j * 128:(j + 1) * 128, :])
        nc.sync.dma_start(wo_sb[:], wo[:])
        make_identity(nc, ident_sb[:])
        # ones columns of v_ext: each 130-wide k-tile block is two 65-wide
        # head blocks [V(64) | ones], so ones sit at col 64 of each half
        vext_g = vext_sb[:].rearrange("p (i two c) -> p i two c", two=2, c=65)
        nc.vector.memset(vext_g[:, :, :, 64:65], 1.0)

        # ---- projections -------------------------------------------------
        with tc.tile_pool(name="xp", bufs=4) as xp, \
             tc.tile_pool(name="pj_ps", bufs=2, space="PSUM") as pj_ps:
            for dst, w_sb, x_dram in ((qt_sb, wq_sb, xq_t),
                                      (kt_sb, wk_sb, xk_t),
                                      (vt_sb, wv_sb, xv_t)):
                for tch in range(T // TCH):
                    ps = pj_ps.tile([128, TCH], F32, tag="pj")
                    for j in range(KT):
                        xt = xp.tile([128, TCH], BF16, tag="x")
                        nc.sync.dma_start(
                            xt[:],
                            x_dram[j * 128:(j + 1) * 128,
                                   tch * TCH:(tch + 1) * TCH])
                        nc.tensor.matmul(ps[:], lhsT=w_sb[:, j * HD:(j + 1) * HD],
                                         rhs=xt[:], start=(j == 0),
                                         stop=(j == KT - 1))
                    nc.vector.tensor_copy(dst[:, tch * TCH:(tch + 1) * TCH],
                                          ps[:])

        # ---- V transpose to natural [token, d] layout --------------------
        with tc.tile_pool(name="vt_ps", bufs=2, space="PSUM") as vt_ps:
            for i in range(T // 128):
                pst = vt_ps.tile([128, 128], BF16, tag="vt")
                nc.tensor.transpose(pst[:], vt_sb[:, i * 128:(i + 1) * 128],
                                    ident_sb[:])
                base = i * VEXT_W
                nc.vector.tensor_copy(vext_sb[:, base:base + 64],
                                      pst[:, 0:64])
                nc.vector.tensor_copy(vext_sb[:, base + 65:base + 129],
                                      pst[:, 64:128])

        # ---- attention: scores -> exp -> AV ------------------------------
        with tc.tile_pool(name="sc_ps", bufs=4, space="PSUM") as sc_ps, \
             tc.tile_pool(name="av_ps", bufs=2, space="PSUM") as av_ps, \
             tc.tile_pool(name="pp", bufs=6) as pp, \
             tc.tile_pool(name="ev", bufs=2) as ev:
            for b in range(B):
                for qc in range(S // QC):
                    col = b * S + qc * QC
                    psA = av_ps.tile([65, QC], F32, tag="avA")
                    psB = av_ps.tile([65, QC], F32, tag="avB")
                    for i in range(NKT):
                        kcol = b * S + i * 128
                        sA = sc_ps.tile([128, QC], F32, tag="sc")
                        sB = sc_ps.tile([128, QC], F32, tag="sc")
                        nc.tensor.matmul(sA[:], lhsT=kt_sb[0:64, kcol:kcol + 128],
                                         rhs=qt_sb[0:64, col:col + QC],
                                         start=True, stop=True)
                        nc.tensor.matmul(sB[:], lhsT=kt_sb[64:128, kcol:kcol + 128],
                                         rhs=qt_sb[64:128, col:col + QC],
                                         start=True, stop=True)
                        pA = pp.tile([128, QC], BF16, tag="p")
                        pB = pp.tile([128, QC], BF16, tag="p")
                        nc.scalar.activation(pA[:], sA[:],
                                             mybir.ActivationFunctionType.Exp,
                                             scale=float(SCALE))
                        nc.scalar.activation(pB[:], sB[:],
                                             mybir.ActivationFunctionType.Exp,
                                             scale=float(SCALE))
                        vbase = (b * NKT + i) * VEXT_W
                        nc.tensor.matmul(psA[:], lhsT=vext_sb[:, vbase:vbase + 65],
                                         rhs=pA[:], start=(i == 0),
                                         stop=(i == NKT - 1))
                        nc.tensor.matmul(psB[:],
                                         lhsT=vext_sb[:, vbase + 65:vbase + 130],
                                         rhs=pB[:], start=(i == 0),
                                         stop=(i == NKT - 1))
                    # psum rows 0-63 = O_unnorm, row 64 = softmax denom.
                    # Normalize here: denom row -> partition 0 (DMA shift),
                    # reciprocal, partition-broadcast to 64 rows, multiply.
                    for hh, ps in (("A", psA), ("B", psB)):
                        dt = ev.tile([65, QC], F32, tag="dt" + hh)
                        nc.vector.tensor_copy(dt[64:65, :], ps[64:65, :])
                        dn = ev.tile([1, QC], F32, tag="dn" + hh)
                        nc.sync.dma_start(dn[0:1, :], dt[64:65, :])
                        rc = ev.tile([1, QC], F32, tag="rc" + hh)
                        nc.vector.reciprocal(rc[0:1, :], dn[0:1, :])
                        rb = ev.tile([64, QC], F32, tag="rb" + hh)
                        nc.gpsimd.partition_broadcast(rb[0:64, :], rc[0:1, :],
                                                      channels=64)
                        if hh == "A":
                            nc.vector.tensor_mul(ot_sb[0:64, col:col + QC],
                                                 ps[0:64, :], rb[0:64, :])
                        else:
                            bt = ev.tile([64, QC], BF16, tag="bt")
                            nc.vector.tensor_mul(bt[0:64, :], ps[0:64, :],
                                                 rb[0:64, :])
                            nc.sync.dma_start(ot_sb[64:128, col:col + QC],
                                              bt[0:64, :])

        # ---- output projection ------------------------------------------
        with tc.tile_pool(name="o5_ps", bufs=3, space="PSUM") as o5_ps, \
             tc.tile_pool(name="op", bufs=3) as op:
            for t in range(T // 128):
                for c in range(D // 512):
                    pso = o5_ps.tile([128, 512], F32, tag="o5")
                    nc.tensor.matmul(pso[:],
                                     lhsT=ot_sb[:, t * 128:(t + 1) * 128],
                                     rhs=wo_sb[:, c * 512:(c + 1) * 512],
                                     start=True, stop=True)
                    ost = op.tile([128, 512], F32, tag="o")
                    nc.vector.tensor_copy(ost[:], pso[:])
                    nc.sync.dma_start(
                        out_p[t * 128:(t + 1) * 128, c * 512:(c + 1) * 512],
                        ost[:])

    nc.compile()
    return nc


def make_in_maps(query, key, value, W_q, W_k, W_v, W_o):
    def xT(x):
        return np.ascontiguousarray(
            np.asarray(x, np.float32).reshape(T, D).astype(NP_BF16).T)

    xq, xk, xv = xT(query), xT(key), xT(value)
    W_q = np.asarray(W_q, np.float32)
    W_k = np.asarray(W_k, np.float32)
    W_v = np.asarray(W_v, np.float32)
    W_o = np.asarray(W_o, np.float32)
    in_maps = []
    for m in range(N_CORES):
        r = slice(m * HD, (m + 1) * HD)
        in_maps.append({
            "xq_t": xq, "xk_t": xk, "xv_t": xv,
            "wq_t": np.ascontiguousarray(W_q[r, :].T).astype(NP_BF16),
            "wk_t": np.ascontiguousarray(W_k[r, :].T).astype(NP_BF16),
            "wv_t": np.ascontiguousarray(W_v[r, :].T).astype(NP_BF16),
            "wo": np.ascontiguousarray(W_o[:, r].T).astype(NP_BF16),
        })
    return in_maps


_NC_CACHE = None


def get_nc():
    global _NC_CACHE
    if _NC_CACHE is None:
        _NC_CACHE = build_nc()
    return _NC_CACHE


def kernel(query, key, value, W_q, W_k, W_v, W_o):
    nc = get_nc()
    in_maps = make_in_maps(query, key, value, W_q, W_k, W_v, W_o)
    res = run_bass_kernel_spmd(nc, in_maps, core_ids=list(range(N_CORES)))
    acc = np.zeros((T, D), np.float32)
    for m in range(N_CORES):
        acc += res.results[m]["out_p"]
    return acc.reshape(B, S, D)
